# revision 37
# baseline (speedup 1.0000x reference)
"""Trainium2 Bass kernel for GQA causal attention block (B=2,T=2048,D=2048,H=16,G=4).

Sharding: 8 cores = batch(2) x kv-group(4). Core c handles batch b=c//4 and
kv-group g=c%4 (query heads 4g..4g+3, which share that kv group). Each core
computes a partial output y_g @ Wo[g-rows] for its batch; the host sums the 4
group partials per batch (bf16 partials, f32 host accumulation).

v2 schedule: PE is the roofline engine (~197.6us of matmul at bf16 peak; fp8
DoubleRow would halve it but e4m3 on any single matmul costs 3.3-4.1e-2 rel
err vs the 2e-2 budget), so emission order is arranged to keep PE dense:
  - x is pre-transposed on the host; xT streams in as [128,512] sub-chunks,
    token-slice-major with weight quads woven in demand order (V, Q-pairs, K),
    so slice-0 projections start at ~2.5us.
  - V q0, K s0, Q s0 are emitted first; attention j starts as soon as slice j
    inputs are roped (~20us for j=0 vs ~85us in v1).
  - Q/K/V slice j+1 projections and wo_stage(j-1) units are emitted as PE
    filler interleaved per-head into attention j (the attention inner loop is
    ACT/exp-bound; the Tile list scheduler slides filler matmuls into the
    exp-wait bubbles, within its ~4-deep stalled-instruction bypass window).
  - the whole denominator tree lives on Pool so each pt tile's reader set is
    {PE, Pool} — this minimizes multi-sem waits, which lower to walrus
    single-wait NOPs that cost real HW dispatch time but are invisible to
    the Tile cost model. Filler wo copies are all-DVE for the same reason.
  - output written bf16 via fat row-block DMAs; the tail wo stage pipelines
    po tiles across all 8 PSUM banks (attention rings are idle by then).
"""

import sys
from contextlib import ExitStack

import numpy as np

sys.path.insert(0, "/opt/trn_rl_repo")

import ml_dtypes

import bass_rust
import concourse.bass as bass
import concourse.mybir as mybir
import concourse.tile as tile
from concourse.bass_utils import run_bass_kernel_spmd

B, T, D = 2, 2048, 2048
H, G, DK = 16, 4, 128
HPC = H // G          # 4 query heads per core
P = 128
NDC = D // P          # 16 contraction chunks
NTB = T // P          # 16 token blocks
QS = 512              # query slice (matmul moving dim)
NQS = T // QS         # 4
ND = D // QS          # 4 output column slices
THETA = 10000.0
SCALE = 1.0 / float(np.sqrt(DK))
BF = mybir.dt.bfloat16
F32 = mybir.dt.float32

_CACHE = {}
_NSPLIT = [0]


def prune_self_waits(nc):
    """Transitive wait pruning: drop a sem wait when it is implied by
    knowledge this instruction already has. Knowledge ("floor" = proven
    minimum per sem) flows from two sound sources:
      1. A kept wait S>=k proves the instruction whose update brought S to
         k COMPLETED (sems increment at completion), so its entire floor
         snapshot is imported.
      2. The same-engine chain: engines execute their stream in order, so
         the floor of the instruction TWO positions back (a >=2 retirement
         margin guards against any pipelined fetch/decode overlap) is
         inherited.
    Each dropped wait is one fewer walrus single-wait NOP — real HW
    dispatch cost that the Tile cost model never sees. DMA sem updates fire
    at transfer completion, which is strictly after trigger dispatch, so
    producer-floor snapshots taken at the trigger remain valid."""
    import collections
    dropped = 0
    for f in nc.m.functions:
        for b in f.blocks:
            # Only the per-engine completion counters are safe to reason
            # about by cumulative count: they are updated solely by one
            # engine's compute instructions and never reset. DMA/HWDGE sems
            # live on finite rings and can wrap — excluded entirely (no
            # pruning, no floor imports).
            upd_engines = collections.defaultdict(set)
            upd_isdma = collections.defaultdict(bool)
            for ins in b.instructions:
                si = getattr(ins, "sync_info", None)
                if si is None:
                    continue
                isdma = ("DMA" in type(ins).__name__
                         or "Trigger" in type(ins).__name__)
                for u in (si.on_update or []):
                    nm = getattr(u, "ant_name", None)
                    if nm is not None:
                        upd_engines[nm].add(ins.engine)
                        upd_isdma[nm] |= isdma
            safe = {nm for nm, engs in upd_engines.items()
                    if len(engs) == 1 and not upd_isdma[nm]}

            sem_count = collections.Counter()
            producer_floor = {}          # (sem, count) -> floor dict snapshot
            eng_hist = collections.defaultdict(list)  # engine -> floors
            for ins in b.instructions:
                si = getattr(ins, "sync_info", None)
                if si is None:
                    continue
                e = ins.engine
                hist = eng_hist[e]
                fl = dict(hist[-2]) if len(hist) >= 2 else {}
                if si.on_wait:
                    keep = []
                    for w in si.on_wait:
                        nm = getattr(w, "ant_name", None)
                        val = getattr(w, "wait_value", None)
                        if (nm in safe and val is not None
                                and getattr(w, "wait_mode", "") == "sem-ge-imm"
                                and fl.get(nm, -1) >= val):
                            dropped += 1
                            continue
                        keep.append(w)
                        if (nm in safe and val is not None
                                and getattr(w, "wait_mode", "") == "sem-ge-imm"):
                            if fl.get(nm, -1) < val:
                                fl[nm] = val
                            pf = producer_floor.get((nm, val))
                            if pf:
                                for s2, v2 in pf.items():
                                    if fl.get(s2, -1) < v2:
                                        fl[s2] = v2
                    if len(keep) != len(si.on_wait):
                        ins.sync_info = mybir.SyncInfo(
                            on_wait=keep, on_update=list(si.on_update or [])
                        )
                for u in (si.on_update or []):
                    nm = getattr(u, "ant_name", None)
                    if nm in safe:
                        sem_count[nm] += 1
                        if fl.get(nm, -1) < sem_count[nm]:
                            fl[nm] = sem_count[nm]
                for u in (si.on_update or []):
                    nm = getattr(u, "ant_name", None)
                    if nm in safe:
                        producer_floor[(nm, sem_count[nm])] = fl
                hist.append(fl)
    return dropped


def split_multi_waits(nc):
    """Walrus codegen accepts at most one sem wait per instruction; Tile's
    sem assignment can emit several. Hoist extras onto single-wait NOPs
    inserted immediately before, on the same engine stream."""
    n = 0
    for f in nc.m.functions:
        for b in f.blocks:
            insts = b.instructions
            newl = []
            changed = False
            for ins in insts:
                si = getattr(ins, "sync_info", None)
                if si is not None and si.on_wait and len(si.on_wait) > 1:
                    waits = list(si.on_wait)
                    for w in waits[:-1]:
                        _NSPLIT[0] += 1
                        nop = bass_rust.InstNoOp(
                            name=f"I-wsplit{_NSPLIT[0]}",
                            engine=ins.engine,
                            ins=[], outs=[],
                            bass_nofuse=True,
                            sync_info=mybir.SyncInfo(on_wait=[w], on_update=[]),
                        )
                        newl.append(nop)
                        n += 1
                    ins.sync_info = mybir.SyncInfo(
                        on_wait=[waits[-1]], on_update=list(si.on_update or [])
                    )
                    changed = True
                newl.append(ins)
            if changed:
                insts.clear()
                insts.extend(newl)
    return n


def build_nc():
    nc = bass.Bass()
    # weights arrive host-pre-rearranged into SBUF layout (partition-major):
    # every DMA is then a fully contiguous per-partition run (>=512B), which
    # avoids the <512B-run 2x DMA latency penalty on wk/wv chunk loads.
    xT = nc.declare_dram_parameter("xT", [D, T], BF, isOutput=False)
    wq = nc.declare_dram_parameter("wq", [P, NDC * HPC * DK], BF, isOutput=False)
    wk = nc.declare_dram_parameter("wk", [P, NDC * DK], BF, isOutput=False)
    wv = nc.declare_dram_parameter("wv", [P, NDC * DK], BF, isOutput=False)
    wo = nc.declare_dram_parameter("wo", [P, HPC * D], BF, isOutput=False)
    cosf = nc.declare_dram_parameter("cosf", [P, T], BF, isOutput=False)
    sinf = nc.declare_dram_parameter("sinf", [P, T], BF, isOutput=False)
    dmask = nc.declare_dram_parameter("dmask", [P, HPC * QS], BF, isOutput=False)
    out = nc.declare_dram_parameter("out", [T, D], BF, isOutput=True)

    with ExitStack() as ctx:
        tc = ctx.enter_context(tile.TileContext(nc))
        const = ctx.enter_context(tc.tile_pool(name="const", bufs=1))
        work = ctx.enter_context(tc.tile_pool(name="work", bufs=3))
        ptp = ctx.enter_context(tc.tile_pool(name="ptp", bufs=10))
        posb = ctx.enter_context(tc.tile_pool(name="posb", bufs=2))
        pst = ctx.enter_context(tc.tile_pool(name="pst", bufs=3, space="PSUM"))
        pyt = ctx.enter_context(tc.tile_pool(name="pyt", bufs=2, space="PSUM"))
        pden = ctx.enter_context(tc.tile_pool(name="pden", bufs=1, space="PSUM"))
        pmm = ctx.enter_context(tc.tile_pool(name="pmm", bufs=2, space="PSUM"))

        # ---- persistent SBUF tiles ----
        xT_sb = const.tile([P, NDC, T], BF, tag="xT")
        wq_sb = const.tile([P, NDC, HPC * DK], BF, tag="wq")
        wk_sb = const.tile([P, NDC, DK], BF, tag="wk")
        wv_sb = const.tile([P, NDC, DK], BF, tag="wv")
        wo_sb = const.tile([P, HPC, D], BF, tag="wo")
        cos_sb = const.tile([P, T], BF, tag="cos")
        sin_sb = const.tile([P, T], BF, tag="sin")
        mask_sb = const.tile([P, HPC, QS], BF, tag="mask")
        ones_sb = const.tile([P, P], BF, tag="ones")
        QT = const.tile([P, HPC, T], BF, tag="QT")
        KT = const.tile([P, T], BF, tag="KT")
        Vn = const.tile([P, NTB, DK], BF, tag="Vn")

        xr = xT.rearrange("(o p) t -> p o t", p=P)
        wq_r = wq.rearrange("p (o m) -> p o m", m=HPC * DK)
        wk_r = wk.rearrange("p (o m) -> p o m", m=DK)
        wv_r = wv.rearrange("p (o m) -> p o m", m=DK)

        # ---- DMA issue (order = queue priority) ----
        # token-slice 0 of xT streams in [128,512] sub-chunks alternating
        # across the two HWDGE queues, weight quads interleaved, so the
        # slice-0 projections (emitted first) are fed from ~2.5us on.
        # Slices 1-3 aren't consumed until ~25us+, so they load as three fat
        # DMAs (fewer descriptors, less queue-floor overhead).
        sl = slice(0, QS)
        for o in range(NDC):
            q = nc.sync if o % 2 == 0 else nc.scalar
            # weight streams woven between the first token-slice chunks,
            # earliest-demand first: V unblocks at the first chunk, the Q
            # chains (4x per-chunk PE demand) right behind via wq pairs,
            # K last.
            if o == 0:
                nc.sync.dma_start(xT_sb[:, 0, sl], xr[:, 0, sl])
                nc.scalar.dma_start(wv_sb[:, 0:4, :], wv_r[:, 0:4, :])
                continue
            sync_pre = {
                2: (wq_sb, wq_r, 0, 2), 4: (wq_sb, wq_r, 2, 4),
                6: (wk_sb, wk_r, 0, 4), 8: (wq_sb, wq_r, 4, 8),
                10: (wk_sb, wk_r, 4, 8), 12: (wq_sb, wq_r, 8, 16),
                14: (wk_sb, wk_r, 8, 16),
            }
            scalar_pre = {
                5: (wv_sb, wv_r, 4, 8), 9: (wv_sb, wv_r, 8, 12),
                13: (wv_sb, wv_r, 12, 16),
            }
            pre = sync_pre.get(o) if o % 2 == 0 else scalar_pre.get(o)
            if pre is not None:
                dst, srcr, a, b = pre
                q.dma_start(dst[:, a:b, :], srcr[:, a:b, :])
            q.dma_start(xT_sb[:, o, sl], xr[:, o, sl])
        nc.scalar.dma_start(cos_sb[:], cosf[:])
        nc.scalar.dma_start(sin_sb[:], sinf[:])
        nc.sync.dma_start(xT_sb[:, :, QS:2 * QS], xr[:, :, QS:2 * QS])
        nc.scalar.dma_start(mask_sb[:], dmask.rearrange("p (d q) -> p d q", q=QS))
        nc.scalar.dma_start(xT_sb[:, :, 2 * QS:3 * QS], xr[:, :, 2 * QS:3 * QS])
        nc.sync.dma_start(wo_sb[:], wo.rearrange("p (h n) -> p h n", n=D))
        nc.sync.dma_start(xT_sb[:, :, 3 * QS:], xr[:, :, 3 * QS:])
        nc.vector.memset(ones_sb[:], 1.0)
        # zero-init the pt pool slots: diagonal blocks only exp the unmasked
        # columns, and mask*stale-NaN would poison the sums otherwise
        for i in range(10):
            ptz = ptp.tile([P, QS], BF, tag="pt", name=f"ptz{i}")
            nc.gpsimd.memset(ptz[:], 0.0)

        # ---- projections ----
        # phase A (K s0, V q0, Q s0) rotates over the attention PSUM rings
        # (idle until ~20us); later slices run as attention filler on the
        # shared "mm" ring.
        _pa = [(pst, "st"), (pst, "st"), (pyt, "yt"),
               (pyt, "yt"), (pden, "den"), (pmm, "mm")]
        _pai = [0]

        def proj_psum(nm):
            if _pai[0] < len(_pa):
                pool, tg = _pa[_pai[0]]
                _pai[0] += 1
            else:
                pool, tg = pmm, "mm"
            return pool.tile([P, QS], F32, tag=tg, name=f"pj_{nm}")

        def rope_slice(dst, ts, nm):  # dst: [128, 512] bf16 AP, token slice ts
            sl = slice(ts * QS, (ts + 1) * QS)
            sw = work.tile([P, QS], BF, tag="swp", name=f"sw{nm}")
            nc.gpsimd.dma_start(sw[0:64, :], dst[64:128, :])
            nc.gpsimd.dma_start(sw[64:128, :], dst[0:64, :])
            nc.vector.tensor_mul(sw[:], sw[:], sin_sb[:, sl])
            nc.vector.tensor_mul(dst, dst, cos_sb[:, sl])
            nc.vector.tensor_add(dst, dst, sw[:])

        def proj_q(h, ts):
            sl = slice(ts * QS, (ts + 1) * QS)
            ps = proj_psum(f"q{h}_{ts}")
            for o in range(NDC):
                nc.tensor.matmul(
                    ps[:],
                    wq_sb[:, o, h * DK:(h + 1) * DK],
                    xT_sb[:, o, sl],
                    start=(o == 0), stop=(o == NDC - 1),
                )
            nc.vector.tensor_copy(QT[:, h, sl], ps[:])
            rope_slice(QT[:, h, sl], ts, f"q{h}_{ts}")

        def proj_k(ts):
            sl = slice(ts * QS, (ts + 1) * QS)
            ps = proj_psum(f"k{ts}")
            for o in range(NDC):
                nc.tensor.matmul(
                    ps[:], wk_sb[:, o, :], xT_sb[:, o, sl],
                    start=(o == 0), stop=(o == NDC - 1),
                )
            nc.vector.tensor_copy(KT[:, sl], ps[:])
            rope_slice(KT[:, sl], ts, f"k{ts}")

        def proj_v_quad(jq):
            # 4 token blocks packed side-by-side into one psum bank; one copy
            ps = proj_psum(f"v{jq}")
            for i in range(4):
                tb = 4 * jq + i
                for o in range(NDC):
                    nc.tensor.matmul(
                        ps[:, i * DK:(i + 1) * DK],
                        xT_sb[:, o, tb * P:(tb + 1) * P], wv_sb[:, o, :],
                        start=(o == 0), stop=(o == NDC - 1),
                    )
            nc.vector.tensor_copy(Vn[:, 4 * jq:4 * jq + 4, :], ps[:])

        ysbs = {}

        # tail wo rotation: attention rings are idle after j=3, so the last
        # wo stage pipelines its po tiles across all 8 PSUM banks (yt/den
        # last — they drain latest).
        _tailrot = [(pst, "st"), (pst, "st"), (pst, "st"), (pmm, "mm"),
                    (pmm, "mm"), (pden, "den"), (pyt, "yt"), (pyt, "yt")]
        _tri = [0]

        def wo_unit(j, tqb, dve_only=False, tail=False):
            # out rows [j*512 + tqb*128, +128): 4 ds column-slices of matmul
            # + copy (alternating DVE/ACT unless ACT is exp-saturated), then
            # fat bf16 row DMAs (halves, so the copy pipeline overlaps).
            r0 = j * QS + tqb * P
            osb = posb.tile([P, D], BF, tag="osb", name=f"osb{j}_{tqb}")
            for ds in range(ND):
                if tail:
                    pool, tg = _tailrot[_tri[0] % 8]
                    _tri[0] += 1
                else:
                    pool, tg = pmm, "mm"
                po = pool.tile([P, QS], F32, tag=tg, name=f"po{j}_{tqb}_{ds}")
                if tail and ds == ND - 1 and tqb == NQS - 1:
                    # span-critical last group: two half-width accumulation
                    # groups in the same bank, so the first half's copy
                    # overlaps the second half's matmuls and the final
                    # copy on the chain is only 256 wide
                    hw_ = QS // 2
                    for half in range(2):
                        c = ds * QS + half * hw_
                        for h in range(HPC):
                            nc.tensor.matmul(
                                po[:, half * hw_:(half + 1) * hw_],
                                ysbs[j][:, h, tqb * P:(tqb + 1) * P],
                                wo_sb[:, h, c:c + hw_],
                                start=(h == 0), stop=(h == HPC - 1),
                            )
                        nc.vector.tensor_copy(
                            osb[:, c:c + hw_], po[:, half * hw_:(half + 1) * hw_])
                    nc.sync.dma_start(out[r0:r0 + P, 2 * QS:3 * QS],
                                      osb[:, 2 * QS:3 * QS])
                    nc.sync.dma_start(out[r0:r0 + P, 3 * QS:],
                                      osb[:, 3 * QS:])
                    continue
                for h in range(HPC):
                    nc.tensor.matmul(
                        po[:],
                        ysbs[j][:, h, tqb * P:(tqb + 1) * P],
                        wo_sb[:, h, ds * QS:(ds + 1) * QS],
                        start=(h == 0), stop=(h == HPC - 1),
                    )
                if dve_only or (tqb + ds) % 2 == 0:
                    nc.vector.tensor_copy(osb[:, ds * QS:(ds + 1) * QS], po[:])
                else:
                    nc.scalar.copy(osb[:, ds * QS:(ds + 1) * QS], po[:])
                if tail and ds % 2 == 1:
                    if ds == ND - 1 and tqb == NQS - 1:
                        nc.sync.dma_start(out[r0:r0 + P, 2 * QS:3 * QS],
                                          osb[:, 2 * QS:3 * QS])
                        nc.sync.dma_start(out[r0:r0 + P, 3 * QS:],
                                          osb[:, 3 * QS:])
                    else:
                        c0 = (ds - 1) * QS
                        nc.sync.dma_start(out[r0:r0 + P, c0:c0 + 2 * QS],
                                          osb[:, c0:c0 + 2 * QS])
            if not tail:
                nc.sync.dma_start(out[r0:r0 + P, :], osb[:])

        def attn_head(j, h, ysb):
            nkb = 4 * j + 4  # causal: key blocks 0..4j+3
            yt = pyt.tile([P, QS], F32, tag="yt", name=f"yt{j}_{h}")
            den = pden.tile([P, QS], F32, tag="den", name=f"den{j}_{h}")
            prev_pt = None
            prev_pts = None
            ptot = None
            for tkb in range(nkb):
                d = tkb - 4 * j
                # columns left of 128*d are fully masked for diagonal
                # blocks: skip them in QK/exp/AV; the mask-mult zeroes
                # the stale region of pt so den/AV sums stay exact.
                c0 = max(d, 0) * P
                st = pst.tile([P, QS], F32, tag="st", name=f"st{j}_{h}_{tkb}")
                nc.tensor.matmul(
                    st[:, c0:],
                    KT[:, tkb * P:(tkb + 1) * P],
                    QT[:, h, j * QS + c0:(j + 1) * QS],
                    start=True, stop=True,
                )
                pt = ptp.tile([P, QS], BF, tag="pt", name=f"pt{j}_{h}_{tkb}")
                nc.scalar.activation(
                    pt[:, c0:], st[:, c0:],
                    mybir.ActivationFunctionType.Exp, scale=SCALE,
                )
                if d >= 0:
                    nc.gpsimd.tensor_mul(pt[:], pt[:], mask_sb[:, d, :])
                nc.tensor.matmul(
                    yt[:, c0:], Vn[:, tkb, :], pt[:, c0:],
                    start=(tkb == 0), stop=(tkb == nkb - 1),
                )
                # denominator tree on Pool (keeps the pt tile's reader set
                # to {PE, Pool}, minimizing exp sem fan-in): pair, quad,
                # chain sums, then one ones-matmul per (h, j)
                if tkb % 2 == 0:
                    prev_pt = pt
                else:
                    pts = ptp.tile([P, QS], BF, tag="pts",
                                   name=f"pts{j}_{h}_{tkb}", bufs=6)
                    nc.gpsimd.tensor_add(pts[:], prev_pt[:], pt[:])
                    if tkb % 4 == 1:
                        prev_pts = pts
                    else:
                        ptq = ptp.tile([P, QS], BF, tag="ptq",
                                       name=f"ptq{j}_{h}_{tkb}", bufs=3)
                        nc.gpsimd.tensor_add(ptq[:], prev_pts[:], pts[:])
                        if ptot is None:
                            ptot = ptq
                        else:
                            nxt = ptp.tile([P, QS], BF, tag="ptt",
                                           name=f"ptt{j}_{h}_{tkb}", bufs=3)
                            nc.gpsimd.tensor_add(nxt[:], ptot[:], ptq[:])
                            ptot = nxt
            nc.tensor.matmul(den[:], ones_sb[:], ptot[:], start=True, stop=True)
            recipb = work.tile([P, QS], F32, tag="recipb", name=f"rb{j}_{h}")
            nc.vector.reciprocal(recipb[:], den[:])
            nc.vector.tensor_mul(ysb[:, h, :], yt[:], recipb[:])

        # ---- schedule ----
        # V first: the V-quad burns 852ns of PE per xT chunk against a
        # ~500ns/chunk DMA supply, so it absorbs the streaming latency that
        # would stall the 213ns/chunk K projection.
        proj_v_quad(0)
        proj_k(0)
        for h in range(HPC):
            proj_q(h, 0)

        for j in range(NQS):
            filler = []
            if j < NQS - 1:
                filler.append(lambda ts=j + 1: proj_k(ts))
                filler.append(lambda jq=j + 1: proj_v_quad(jq))
                for h in range(HPC):
                    filler.append(lambda h=h, ts=j + 1: proj_q(h, ts))
            if j >= 1:
                # filler wo copies all-DVE: keeps the exp-saturated ACT
                # engine out of the wo pipeline (and out of the osb tile's
                # reader set, reducing cross-engine sem fan-in)
                for tqb in range(NQS):
                    filler.append(lambda jj=j - 1, tqb=tqb:
                                  wo_unit(jj, tqb, dve_only=True))
            ysb = work.tile([P, HPC, QS], BF, tag="ysb", name=f"ysb{j}")
            ysbs[j] = ysb
            per = (len(filler) + HPC - 1) // HPC if filler else 0
            for h in range(HPC):
                attn_head(j, h, ysb)
                for f in filler[h * per:(h + 1) * per]:
                    f()
        for tqb in range(NQS):
            wo_unit(NQS - 1, tqb, tail=True)
    prune_self_waits(nc)
    split_multi_waits(nc)
    return nc


def _rope_tables(pos):
    inv_freq = 1.0 / (THETA ** (np.arange(0, DK // 2, dtype=np.float64) * 2.0 / DK))
    ang = pos.astype(np.float64)[:, None] * inv_freq[None, :]   # (T, 64)
    cos = np.cos(ang).T.astype(np.float32)                      # (64, T)
    sin = np.sin(ang).T.astype(np.float32)
    cosf = np.concatenate([cos, cos], axis=0)                   # (128, T)
    sinf = np.concatenate([-sin, sin], axis=0)
    return cosf, sinf


def _make_in_maps(inputs):
    x, Wq, Wk, Wv, Wo = (np.asarray(inputs[k]) for k in
                         ("x", "Wq", "Wk", "Wv", "Wo"))
    bf = ml_dtypes.bfloat16
    cosf, sinf = _rope_tables(np.asarray(inputs["pos"]))
    cosf = cosf.astype(bf)
    sinf = sinf.astype(bf)
    # diagonal-region 0/1 masks in SBUF layout [p, d*q]:
    # dmask[tk, d, tq] = mask[tq, d*128 + tk]
    m = np.asarray(inputs["mask"])
    dmask = np.stack(
        [m[0:QS, d * P:(d + 1) * P].T for d in range(HPC)], axis=1
    ).reshape(P, HPC * QS).astype(bf)

    def pm(w, cols):  # [(o p), m] -> partition-major [p, o*m] contiguous
        return np.ascontiguousarray(
            w.reshape(NDC, P, cols).transpose(1, 0, 2).reshape(P, NDC * cols)
        ).astype(bf)

    in_maps = []
    for c in range(8):
        b, g = c // 4, c % 4
        wos = Wo[g * HPC * DK:(g + 1) * HPC * DK, :]
        in_maps.append({
            "xT": np.ascontiguousarray(x[b].T).astype(bf),
            "wq": pm(Wq[:, g * HPC * DK:(g + 1) * HPC * DK], HPC * DK),
            "wk": pm(Wk[:, g * DK:(g + 1) * DK], DK),
            "wv": pm(Wv[:, g * DK:(g + 1) * DK], DK),
            "wo": np.ascontiguousarray(
                wos.reshape(HPC, P, D).transpose(1, 0, 2).reshape(P, HPC * D)
            ).astype(bf),
            "cosf": cosf, "sinf": sinf, "dmask": dmask,
        })
    return in_maps


def kernel(x, Wq, Wk, Wv, Wo, mask, pos):
    in_maps = _make_in_maps(dict(x=x, Wq=Wq, Wk=Wk, Wv=Wv, Wo=Wo,
                                 mask=mask, pos=pos))
    if "nc" not in _CACHE:
        _CACHE["nc"] = build_nc()
    nc = _CACHE["nc"]

    res = run_bass_kernel_spmd(nc, in_maps, core_ids=list(range(8)))
    outs = [np.asarray(r["out"], dtype=np.float32) for r in res.results]
    full = np.stack([
        outs[0] + outs[1] + outs[2] + outs[3],
        outs[4] + outs[5] + outs[6] + outs[7],
    ]).astype(np.float32)
    return full


# revision 38
# speedup vs baseline: 1.0016x; 1.0016x over previous
"""Trainium2 Bass kernel for GQA causal attention block (B=2,T=2048,D=2048,H=16,G=4).

Sharding: 8 cores = batch(2) x kv-group(4). Core c handles batch b=c//4 and
kv-group g=c%4 (query heads 4g..4g+3, which share that kv group). Each core
computes a partial output y_g @ Wo[g-rows] for its batch; the host sums the 4
group partials per batch (bf16 partials, f32 host accumulation).

v2 schedule: PE is the roofline engine (~197.6us of matmul at bf16 peak; fp8
DoubleRow would halve it but e4m3 on any single matmul costs 3.3-4.1e-2 rel
err vs the 2e-2 budget), so emission order is arranged to keep PE dense:
  - x is pre-transposed on the host; xT streams in as [128,512] sub-chunks,
    token-slice-major with weight quads woven in demand order (V, Q-pairs, K),
    so slice-0 projections start at ~2.5us.
  - V q0, K s0, Q s0 are emitted first; attention j starts as soon as slice j
    inputs are roped (~20us for j=0 vs ~85us in v1).
  - Q/K/V slice j+1 projections and wo_stage(j-1) units are emitted as PE
    filler interleaved per-head into attention j (the attention inner loop is
    ACT/exp-bound; the Tile list scheduler slides filler matmuls into the
    exp-wait bubbles, within its ~4-deep stalled-instruction bypass window).
  - the whole denominator tree lives on Pool so each pt tile's reader set is
    {PE, Pool} — this minimizes multi-sem waits, which lower to walrus
    single-wait NOPs that cost real HW dispatch time but are invisible to
    the Tile cost model. Filler wo copies are all-DVE for the same reason.
  - output written bf16 via fat row-block DMAs; the tail wo stage pipelines
    po tiles across all 8 PSUM banks (attention rings are idle by then).
"""

import sys
from contextlib import ExitStack

import numpy as np

sys.path.insert(0, "/opt/trn_rl_repo")

import ml_dtypes

import bass_rust
import concourse.bass as bass
import concourse.mybir as mybir
import concourse.tile as tile
from concourse.bass_utils import run_bass_kernel_spmd

B, T, D = 2, 2048, 2048
H, G, DK = 16, 4, 128
HPC = H // G          # 4 query heads per core
P = 128
NDC = D // P          # 16 contraction chunks
NTB = T // P          # 16 token blocks
QS = 512              # query slice (matmul moving dim)
NQS = T // QS         # 4
ND = D // QS          # 4 output column slices
THETA = 10000.0
SCALE = 1.0 / float(np.sqrt(DK))
BF = mybir.dt.bfloat16
F32 = mybir.dt.float32

_CACHE = {}
_NSPLIT = [0]


def prune_self_waits(nc):
    """Transitive wait pruning: drop a sem wait when it is implied by
    knowledge this instruction already has. Knowledge ("floor" = proven
    minimum per sem) flows from two sound sources:
      1. A kept wait S>=k proves the instruction whose update brought S to
         k COMPLETED (sems increment at completion), so its entire floor
         snapshot is imported.
      2. The same-engine chain: engines execute their stream in order, so
         the floor of the instruction TWO positions back (a >=2 retirement
         margin guards against any pipelined fetch/decode overlap) is
         inherited.
    Each dropped wait is one fewer walrus single-wait NOP — real HW
    dispatch cost that the Tile cost model never sees. DMA sem updates fire
    at transfer completion, which is strictly after trigger dispatch, so
    producer-floor snapshots taken at the trigger remain valid."""
    import collections
    dropped = 0
    for f in nc.m.functions:
        for b in f.blocks:
            # Only the per-engine completion counters are safe to reason
            # about by cumulative count: they are updated solely by one
            # engine's compute instructions and never reset. DMA/HWDGE sems
            # live on finite rings and can wrap — excluded entirely (no
            # pruning, no floor imports).
            upd_engines = collections.defaultdict(set)
            upd_isdma = collections.defaultdict(bool)
            for ins in b.instructions:
                si = getattr(ins, "sync_info", None)
                if si is None:
                    continue
                isdma = ("DMA" in type(ins).__name__
                         or "Trigger" in type(ins).__name__)
                for u in (si.on_update or []):
                    nm = getattr(u, "ant_name", None)
                    if nm is not None:
                        upd_engines[nm].add(ins.engine)
                        upd_isdma[nm] |= isdma
            safe = {nm for nm, engs in upd_engines.items()
                    if len(engs) == 1 and not upd_isdma[nm]}

            sem_count = collections.Counter()
            producer_floor = {}          # (sem, count) -> floor dict snapshot
            eng_hist = collections.defaultdict(list)  # engine -> floors
            for ins in b.instructions:
                si = getattr(ins, "sync_info", None)
                if si is None:
                    continue
                e = ins.engine
                hist = eng_hist[e]
                fl = dict(hist[-2]) if len(hist) >= 2 else {}
                if si.on_wait:
                    keep = []
                    for w in si.on_wait:
                        nm = getattr(w, "ant_name", None)
                        val = getattr(w, "wait_value", None)
                        if (nm in safe and val is not None
                                and getattr(w, "wait_mode", "") == "sem-ge-imm"
                                and fl.get(nm, -1) >= val):
                            dropped += 1
                            continue
                        keep.append(w)
                        if (nm in safe and val is not None
                                and getattr(w, "wait_mode", "") == "sem-ge-imm"):
                            if fl.get(nm, -1) < val:
                                fl[nm] = val
                            pf = producer_floor.get((nm, val))
                            if pf:
                                for s2, v2 in pf.items():
                                    if fl.get(s2, -1) < v2:
                                        fl[s2] = v2
                    if len(keep) != len(si.on_wait):
                        ins.sync_info = mybir.SyncInfo(
                            on_wait=keep, on_update=list(si.on_update or [])
                        )
                for u in (si.on_update or []):
                    nm = getattr(u, "ant_name", None)
                    if nm in safe:
                        sem_count[nm] += 1
                        if fl.get(nm, -1) < sem_count[nm]:
                            fl[nm] = sem_count[nm]
                for u in (si.on_update or []):
                    nm = getattr(u, "ant_name", None)
                    if nm in safe:
                        producer_floor[(nm, sem_count[nm])] = fl
                hist.append(fl)
    return dropped


def split_multi_waits(nc):
    """Walrus codegen accepts at most one sem wait per instruction; Tile's
    sem assignment can emit several. Hoist extras onto single-wait NOPs
    inserted immediately before, on the same engine stream."""
    n = 0
    for f in nc.m.functions:
        for b in f.blocks:
            insts = b.instructions
            newl = []
            changed = False
            for ins in insts:
                si = getattr(ins, "sync_info", None)
                if si is not None and si.on_wait and len(si.on_wait) > 1:
                    waits = list(si.on_wait)
                    for w in waits[:-1]:
                        _NSPLIT[0] += 1
                        nop = bass_rust.InstNoOp(
                            name=f"I-wsplit{_NSPLIT[0]}",
                            engine=ins.engine,
                            ins=[], outs=[],
                            bass_nofuse=True,
                            sync_info=mybir.SyncInfo(on_wait=[w], on_update=[]),
                        )
                        newl.append(nop)
                        n += 1
                    ins.sync_info = mybir.SyncInfo(
                        on_wait=[waits[-1]], on_update=list(si.on_update or [])
                    )
                    changed = True
                newl.append(ins)
            if changed:
                insts.clear()
                insts.extend(newl)
    return n


def build_nc():
    nc = bass.Bass()
    # weights arrive host-pre-rearranged into SBUF layout (partition-major):
    # every DMA is then a fully contiguous per-partition run (>=512B), which
    # avoids the <512B-run 2x DMA latency penalty on wk/wv chunk loads.
    xT = nc.declare_dram_parameter("xT", [D, T], BF, isOutput=False)
    wq = nc.declare_dram_parameter("wq", [P, NDC * HPC * DK], BF, isOutput=False)
    wk = nc.declare_dram_parameter("wk", [P, NDC * DK], BF, isOutput=False)
    wv = nc.declare_dram_parameter("wv", [P, NDC * DK], BF, isOutput=False)
    wo = nc.declare_dram_parameter("wo", [P, HPC * D], BF, isOutput=False)
    cosf = nc.declare_dram_parameter("cosf", [P, T], BF, isOutput=False)
    sinf = nc.declare_dram_parameter("sinf", [P, T], BF, isOutput=False)
    dmask = nc.declare_dram_parameter("dmask", [P, HPC * QS], BF, isOutput=False)
    out = nc.declare_dram_parameter("out", [T, D], BF, isOutput=True)

    with ExitStack() as ctx:
        tc = ctx.enter_context(tile.TileContext(nc))
        const = ctx.enter_context(tc.tile_pool(name="const", bufs=1))
        work = ctx.enter_context(tc.tile_pool(name="work", bufs=3))
        ptp = ctx.enter_context(tc.tile_pool(name="ptp", bufs=10))
        posb = ctx.enter_context(tc.tile_pool(name="posb", bufs=2))
        pst = ctx.enter_context(tc.tile_pool(name="pst", bufs=3, space="PSUM"))
        pyt = ctx.enter_context(tc.tile_pool(name="pyt", bufs=2, space="PSUM"))
        pden = ctx.enter_context(tc.tile_pool(name="pden", bufs=1, space="PSUM"))
        pmm = ctx.enter_context(tc.tile_pool(name="pmm", bufs=2, space="PSUM"))

        # ---- persistent SBUF tiles ----
        xT_sb = const.tile([P, NDC, T], BF, tag="xT")
        wq_sb = const.tile([P, NDC, HPC * DK], BF, tag="wq")
        wk_sb = const.tile([P, NDC, DK], BF, tag="wk")
        wv_sb = const.tile([P, NDC, DK], BF, tag="wv")
        wo_sb = const.tile([P, HPC, D], BF, tag="wo")
        cos_sb = const.tile([P, T], BF, tag="cos")
        sin_sb = const.tile([P, T], BF, tag="sin")
        mask_sb = const.tile([P, HPC, QS], BF, tag="mask")
        ones_sb = const.tile([P, P], BF, tag="ones")
        QT = const.tile([P, HPC, T], BF, tag="QT")
        KT = const.tile([P, T], BF, tag="KT")
        Vn = const.tile([P, NTB, DK], BF, tag="Vn")

        xr = xT.rearrange("(o p) t -> p o t", p=P)
        wq_r = wq.rearrange("p (o m) -> p o m", m=HPC * DK)
        wk_r = wk.rearrange("p (o m) -> p o m", m=DK)
        wv_r = wv.rearrange("p (o m) -> p o m", m=DK)

        # ---- DMA issue (order = queue priority) ----
        # token-slice 0 of xT streams in [128,512] sub-chunks alternating
        # across the two HWDGE queues, weight quads interleaved, so the
        # slice-0 projections (emitted first) are fed from ~2.5us on.
        # Slices 1-3 aren't consumed until ~25us+, so they load as three fat
        # DMAs (fewer descriptors, less queue-floor overhead).
        sl = slice(0, QS)
        for o in range(NDC):
            q = nc.sync if o % 2 == 0 else nc.scalar
            # weight streams woven between the first token-slice chunks,
            # earliest-demand first: V unblocks at the first chunk, the Q
            # chains (4x per-chunk PE demand) right behind via wq pairs,
            # K last.
            if o == 0:
                nc.sync.dma_start(xT_sb[:, 0, sl], xr[:, 0, sl])
                nc.scalar.dma_start(wv_sb[:, 0:4, :], wv_r[:, 0:4, :])
                continue
            sync_pre = {
                2: (wq_sb, wq_r, 0, 2), 4: (wq_sb, wq_r, 2, 4),
                6: (wk_sb, wk_r, 0, 4), 8: (wq_sb, wq_r, 4, 8),
                10: (wk_sb, wk_r, 4, 8), 12: (wq_sb, wq_r, 8, 16),
                14: (wk_sb, wk_r, 8, 16),
            }
            scalar_pre = {
                5: (wv_sb, wv_r, 4, 8), 9: (wv_sb, wv_r, 8, 12),
                13: (wv_sb, wv_r, 12, 16),
            }
            pre = sync_pre.get(o) if o % 2 == 0 else scalar_pre.get(o)
            if pre is not None:
                dst, srcr, a, b = pre
                q.dma_start(dst[:, a:b, :], srcr[:, a:b, :])
            q.dma_start(xT_sb[:, o, sl], xr[:, o, sl])
        nc.scalar.dma_start(cos_sb[:], cosf[:])
        nc.scalar.dma_start(sin_sb[:], sinf[:])
        nc.sync.dma_start(xT_sb[:, :, QS:2 * QS], xr[:, :, QS:2 * QS])
        nc.scalar.dma_start(mask_sb[:], dmask.rearrange("p (d q) -> p d q", q=QS))
        nc.scalar.dma_start(xT_sb[:, :, 2 * QS:3 * QS], xr[:, :, 2 * QS:3 * QS])
        nc.sync.dma_start(wo_sb[:], wo.rearrange("p (h n) -> p h n", n=D))
        nc.sync.dma_start(xT_sb[:, :, 3 * QS:], xr[:, :, 3 * QS:])
        nc.vector.memset(ones_sb[:], 1.0)
        # zero-init the pt pool slots: diagonal blocks only exp the unmasked
        # columns, and mask*stale-NaN would poison the sums otherwise
        for i in range(10):
            ptz = ptp.tile([P, QS], BF, tag="pt", name=f"ptz{i}")
            nc.gpsimd.memset(ptz[:], 0.0)

        # ---- projections ----
        # phase A (K s0, V q0, Q s0) rotates over the attention PSUM rings
        # (idle until ~20us); later slices run as attention filler on the
        # shared "mm" ring.
        _pa = [(pst, "st"), (pst, "st"), (pyt, "yt"),
               (pyt, "yt"), (pden, "den"), (pmm, "mm")]
        _pai = [0]

        def proj_psum(nm):
            if _pai[0] < len(_pa):
                pool, tg = _pa[_pai[0]]
                _pai[0] += 1
            else:
                pool, tg = pmm, "mm"
            return pool.tile([P, QS], F32, tag=tg, name=f"pj_{nm}")

        def rope_slice(dst, ts, nm):  # dst: [128, 512] bf16 AP, token slice ts
            sl = slice(ts * QS, (ts + 1) * QS)
            sw = work.tile([P, QS], BF, tag="swp", name=f"sw{nm}")
            nc.gpsimd.dma_start(sw[0:64, :], dst[64:128, :])
            nc.gpsimd.dma_start(sw[64:128, :], dst[0:64, :])
            nc.vector.tensor_mul(sw[:], sw[:], sin_sb[:, sl])
            nc.vector.tensor_mul(dst, dst, cos_sb[:, sl])
            nc.vector.tensor_add(dst, dst, sw[:])

        def proj_q(h, ts):
            sl = slice(ts * QS, (ts + 1) * QS)
            ps = proj_psum(f"q{h}_{ts}")
            for o in range(NDC):
                nc.tensor.matmul(
                    ps[:],
                    wq_sb[:, o, h * DK:(h + 1) * DK],
                    xT_sb[:, o, sl],
                    start=(o == 0), stop=(o == NDC - 1),
                )
            nc.vector.tensor_copy(QT[:, h, sl], ps[:])
            rope_slice(QT[:, h, sl], ts, f"q{h}_{ts}")

        def proj_k(ts):
            sl = slice(ts * QS, (ts + 1) * QS)
            ps = proj_psum(f"k{ts}")
            for o in range(NDC):
                nc.tensor.matmul(
                    ps[:], wk_sb[:, o, :], xT_sb[:, o, sl],
                    start=(o == 0), stop=(o == NDC - 1),
                )
            nc.vector.tensor_copy(KT[:, sl], ps[:])
            rope_slice(KT[:, sl], ts, f"k{ts}")

        def proj_v_quad(jq):
            # 4 token blocks packed side-by-side into one psum bank; one copy
            ps = proj_psum(f"v{jq}")
            for i in range(4):
                tb = 4 * jq + i
                for o in range(NDC):
                    nc.tensor.matmul(
                        ps[:, i * DK:(i + 1) * DK],
                        xT_sb[:, o, tb * P:(tb + 1) * P], wv_sb[:, o, :],
                        start=(o == 0), stop=(o == NDC - 1),
                    )
            nc.vector.tensor_copy(Vn[:, 4 * jq:4 * jq + 4, :], ps[:])

        ysbs = {}

        # tail wo rotation: attention rings are idle after j=3, so the last
        # wo stage pipelines its po tiles across all 8 PSUM banks (yt/den
        # last — they drain latest).
        _tailrot = [(pst, "st"), (pst, "st"), (pst, "st"), (pmm, "mm"),
                    (pmm, "mm"), (pden, "den"), (pyt, "yt"), (pyt, "yt")]
        _tri = [0]

        def wo_unit(j, tqb, dve_only=False, tail=False):
            # out rows [j*512 + tqb*128, +128): 4 ds column-slices of matmul
            # + copy (alternating DVE/ACT unless ACT is exp-saturated), then
            # fat bf16 row DMAs (halves, so the copy pipeline overlaps).
            r0 = j * QS + tqb * P
            osb = posb.tile([P, D], BF, tag="osb", name=f"osb{j}_{tqb}")
            for ds in range(ND):
                if tail:
                    pool, tg = _tailrot[_tri[0] % 8]
                    _tri[0] += 1
                else:
                    pool, tg = pmm, "mm"
                po = pool.tile([P, QS], F32, tag=tg, name=f"po{j}_{tqb}_{ds}")
                for h in range(HPC):
                    nc.tensor.matmul(
                        po[:],
                        ysbs[j][:, h, tqb * P:(tqb + 1) * P],
                        wo_sb[:, h, ds * QS:(ds + 1) * QS],
                        start=(h == 0), stop=(h == HPC - 1),
                    )
                if dve_only or (tqb + ds) % 2 == 0:
                    nc.vector.tensor_copy(osb[:, ds * QS:(ds + 1) * QS], po[:])
                else:
                    nc.scalar.copy(osb[:, ds * QS:(ds + 1) * QS], po[:])
                if tail and ds % 2 == 1:
                    if ds == ND - 1 and tqb == NQS - 1:
                        nc.sync.dma_start(out[r0:r0 + P, 2 * QS:3 * QS],
                                          osb[:, 2 * QS:3 * QS])
                        nc.sync.dma_start(out[r0:r0 + P, 3 * QS:],
                                          osb[:, 3 * QS:])
                    else:
                        c0 = (ds - 1) * QS
                        nc.sync.dma_start(out[r0:r0 + P, c0:c0 + 2 * QS],
                                          osb[:, c0:c0 + 2 * QS])
            if not tail:
                nc.sync.dma_start(out[r0:r0 + P, :], osb[:])

        def attn_head(j, h, ysb):
            nkb = 4 * j + 4  # causal: key blocks 0..4j+3
            yt = pyt.tile([P, QS], F32, tag="yt", name=f"yt{j}_{h}")
            den = pden.tile([P, QS], F32, tag="den", name=f"den{j}_{h}")
            prev_pt = None
            prev_pts = None
            ptot = None
            for tkb in range(nkb):
                d = tkb - 4 * j
                # columns left of 128*d are fully masked for diagonal
                # blocks: skip them in QK/exp/AV; the mask-mult zeroes
                # the stale region of pt so den/AV sums stay exact.
                c0 = max(d, 0) * P
                st = pst.tile([P, QS], F32, tag="st", name=f"st{j}_{h}_{tkb}")
                nc.tensor.matmul(
                    st[:, c0:],
                    KT[:, tkb * P:(tkb + 1) * P],
                    QT[:, h, j * QS + c0:(j + 1) * QS],
                    start=True, stop=True,
                )
                pt = ptp.tile([P, QS], BF, tag="pt", name=f"pt{j}_{h}_{tkb}")
                nc.scalar.activation(
                    pt[:, c0:], st[:, c0:],
                    mybir.ActivationFunctionType.Exp, scale=SCALE,
                )
                if d >= 0:
                    nc.gpsimd.tensor_mul(pt[:], pt[:], mask_sb[:, d, :])
                nc.tensor.matmul(
                    yt[:, c0:], Vn[:, tkb, :], pt[:, c0:],
                    start=(tkb == 0), stop=(tkb == nkb - 1),
                )
                # denominator tree on Pool (keeps the pt tile's reader set
                # to {PE, Pool}, minimizing exp sem fan-in): pair, quad,
                # chain sums, then one ones-matmul per (h, j)
                if tkb % 2 == 0:
                    prev_pt = pt
                else:
                    pts = ptp.tile([P, QS], BF, tag="pts",
                                   name=f"pts{j}_{h}_{tkb}", bufs=6)
                    nc.gpsimd.tensor_add(pts[:], prev_pt[:], pt[:])
                    if tkb % 4 == 1:
                        prev_pts = pts
                    else:
                        ptq = ptp.tile([P, QS], BF, tag="ptq",
                                       name=f"ptq{j}_{h}_{tkb}", bufs=3)
                        nc.gpsimd.tensor_add(ptq[:], prev_pts[:], pts[:])
                        if ptot is None:
                            ptot = ptq
                        else:
                            nxt = ptp.tile([P, QS], BF, tag="ptt",
                                           name=f"ptt{j}_{h}_{tkb}", bufs=3)
                            nc.gpsimd.tensor_add(nxt[:], ptot[:], ptq[:])
                            ptot = nxt
            nc.tensor.matmul(den[:], ones_sb[:], ptot[:], start=True, stop=True)
            recipb = work.tile([P, QS], F32, tag="recipb", name=f"rb{j}_{h}")
            nc.vector.reciprocal(recipb[:], den[:])
            nc.vector.tensor_mul(ysb[:, h, :], yt[:], recipb[:])

        # ---- schedule ----
        # V first: the V-quad burns 852ns of PE per xT chunk against a
        # ~500ns/chunk DMA supply, so it absorbs the streaming latency that
        # would stall the 213ns/chunk K projection.
        proj_v_quad(0)
        proj_k(0)
        for h in range(HPC):
            proj_q(h, 0)

        for j in range(NQS):
            filler = []
            if j < NQS - 1:
                filler.append(lambda ts=j + 1: proj_k(ts))
                filler.append(lambda jq=j + 1: proj_v_quad(jq))
                for h in range(HPC):
                    filler.append(lambda h=h, ts=j + 1: proj_q(h, ts))
            if j >= 1:
                # filler wo copies all-DVE: keeps the exp-saturated ACT
                # engine out of the wo pipeline (and out of the osb tile's
                # reader set, reducing cross-engine sem fan-in)
                for tqb in range(NQS):
                    filler.append(lambda jj=j - 1, tqb=tqb:
                                  wo_unit(jj, tqb, dve_only=True))
            ysb = work.tile([P, HPC, QS], BF, tag="ysb", name=f"ysb{j}")
            ysbs[j] = ysb
            per = (len(filler) + HPC - 1) // HPC if filler else 0
            for h in range(HPC):
                attn_head(j, h, ysb)
                for f in filler[h * per:(h + 1) * per]:
                    f()
        for tqb in range(NQS):
            wo_unit(NQS - 1, tqb, tail=True)
    prune_self_waits(nc)
    split_multi_waits(nc)
    return nc


def _rope_tables(pos):
    inv_freq = 1.0 / (THETA ** (np.arange(0, DK // 2, dtype=np.float64) * 2.0 / DK))
    ang = pos.astype(np.float64)[:, None] * inv_freq[None, :]   # (T, 64)
    cos = np.cos(ang).T.astype(np.float32)                      # (64, T)
    sin = np.sin(ang).T.astype(np.float32)
    cosf = np.concatenate([cos, cos], axis=0)                   # (128, T)
    sinf = np.concatenate([-sin, sin], axis=0)
    return cosf, sinf


def _make_in_maps(inputs):
    x, Wq, Wk, Wv, Wo = (np.asarray(inputs[k]) for k in
                         ("x", "Wq", "Wk", "Wv", "Wo"))
    bf = ml_dtypes.bfloat16
    cosf, sinf = _rope_tables(np.asarray(inputs["pos"]))
    cosf = cosf.astype(bf)
    sinf = sinf.astype(bf)
    # diagonal-region 0/1 masks in SBUF layout [p, d*q]:
    # dmask[tk, d, tq] = mask[tq, d*128 + tk]
    m = np.asarray(inputs["mask"])
    dmask = np.stack(
        [m[0:QS, d * P:(d + 1) * P].T for d in range(HPC)], axis=1
    ).reshape(P, HPC * QS).astype(bf)

    def pm(w, cols):  # [(o p), m] -> partition-major [p, o*m] contiguous
        return np.ascontiguousarray(
            w.reshape(NDC, P, cols).transpose(1, 0, 2).reshape(P, NDC * cols)
        ).astype(bf)

    in_maps = []
    for c in range(8):
        b, g = c // 4, c % 4
        wos = Wo[g * HPC * DK:(g + 1) * HPC * DK, :]
        in_maps.append({
            "xT": np.ascontiguousarray(x[b].T).astype(bf),
            "wq": pm(Wq[:, g * HPC * DK:(g + 1) * HPC * DK], HPC * DK),
            "wk": pm(Wk[:, g * DK:(g + 1) * DK], DK),
            "wv": pm(Wv[:, g * DK:(g + 1) * DK], DK),
            "wo": np.ascontiguousarray(
                wos.reshape(HPC, P, D).transpose(1, 0, 2).reshape(P, HPC * D)
            ).astype(bf),
            "cosf": cosf, "sinf": sinf, "dmask": dmask,
        })
    return in_maps


def kernel(x, Wq, Wk, Wv, Wo, mask, pos):
    in_maps = _make_in_maps(dict(x=x, Wq=Wq, Wk=Wk, Wv=Wv, Wo=Wo,
                                 mask=mask, pos=pos))
    if "nc" not in _CACHE:
        _CACHE["nc"] = build_nc()
    nc = _CACHE["nc"]

    res = run_bass_kernel_spmd(nc, in_maps, core_ids=list(range(8)))
    outs = [np.asarray(r["out"], dtype=np.float32) for r in res.results]
    full = np.stack([
        outs[0] + outs[1] + outs[2] + outs[3],
        outs[4] + outs[5] + outs[6] + outs[7],
    ]).astype(np.float32)
    return full
